# revision 3
# baseline (speedup 1.0000x reference)
"""Trainium2 Bass kernel: DGCNN Zernike-monomial interwiner (nn_DGCNN_8839042695322).

Computes, per point p=(x,y,z):
  out[.., 16, 4] = concat_l( einsum(zernike_monoms(p)[l], Wl) ) for l=0..3
Every output channel is a degree<=3 polynomial in (x,y,z), so all weights are
folded host-side into per-channel scalars; the device computes shared
polynomial planes and scales them into the output layout.

Sharding: pure data parallel over the batch axis across 8 NeuronCores.
"""

import numpy as np

import concourse.bacc as bacc
import concourse.tile as tile
from concourse import mybir
from concourse.bass_utils import run_bass_kernel_spmd

# Problem geometry (hardcoded per spec: x [32, 32768, 3] f32, 8 cores).
B, N, M_CORES = 32, 32768, 8
PTS_PER_CORE = B * N // M_CORES  # 131072
P = 128                          # SBUF partitions
COLS = PTS_PER_CORE // P         # 1024 points per partition
T = 256                          # points per partition per iteration
N_ITERS = COLS // T              # 4

# Real spherical-harmonic constants (match reference).
C0 = 0.28209479177387814
C1 = 0.4886025119029199
C2_XY = 1.0925484305920792
C2_0 = 0.31539156525252005
C2_2 = 0.5462742152960396
C3_3 = 0.5900435899266435
C3_2 = 2.890611442640554
C3_1 = 0.4570457994644658
C3_0 = 0.3731763325901154
C3_P2 = 1.445305721320277

# wconst layout (per-partition replicated [P, 64]):
#   0..19  w2[m,u]  (m_total 4..8)
#   20..47 w3[m,u]  (m_total 9..15)
#   48..51 A0[u]    52..55 B0[u]   (l=0: out = A0 + B0*n2)
#   56..59 AA1[u]   60..63 BB1[u]  (l=1: s'_u = AA1 + BB1*n2)

# Engine assignment for the 12 "simple" channel groups (m_total -> engine).
# Each group is out[:, m*4+u] = base_plane * w[m,u] for u=0..3.
GROUP_ENGINE = {
    4: "dve",      # xy
    5: "act",      # yz
    6: "dve",      # 2z2-x2-y2
    7: "act",      # xz
    8: "gpsimd",   # x2-y2
    9: "dve",      # y*(3x2-y2)
    10: "act",     # xyz
    11: "dve",     # y*(4z2-x2-y2)
    12: "act",     # z*(2z2-3x2-3y2)
    13: "gpsimd",  # x*(4z2-x2-y2)
    14: "dve",     # z*(x2-y2)
    15: "act",     # x*(x2-3y2)
}

_cache: dict = {}


def _build_program():
    if "nc" in _cache:
        return _cache["nc"]

    dt = mybir.dt.float32
    F = mybir.ActivationFunctionType
    ALU = mybir.AluOpType

    nc = bacc.Bacc(
        "TRN2", target_bir_lowering=False, debug=False, num_devices=M_CORES
    )
    xin = nc.dram_tensor("xin", [P, COLS, 3], dt, kind="ExternalInput").ap()
    wcd = nc.dram_tensor("wconst", [P, 64], dt, kind="ExternalInput").ap()
    yout = nc.dram_tensor("yout", [P, COLS, 64], dt, kind="ExternalOutput").ap()

    with tile.TileContext(nc) as tc:
        with (
            tc.tile_pool(name="singles", bufs=1) as singles,
            tc.tile_pool(name="xpool", bufs=2) as xpool,
            tc.tile_pool(name="planes", bufs=2) as planes,
            tc.tile_pool(name="opool", bufs=2) as opool,
        ):
            wc = singles.tile([P, 64], dt)
            nc.sync.dma_start(out=wc, in_=wcd)

            def wap(i):  # [P,1] per-partition scalar view of wconst
                return wc[:, i : i + 1]

            for it in range(N_ITERS):
                ts = it * T
                xt = xpool.tile([P, T * 3], dt)
                nc.sync.dma_start(
                    out=xt.rearrange("p (t c) -> p t c", c=3),
                    in_=xin[:, ts : ts + T, :],
                )
                xv = xt.rearrange("p (t c) -> p t c", c=3)
                px, py, pz = xv[:, :, 0], xv[:, :, 1], xv[:, :, 2]

                # allocate named planes (distinct tags so each gets a slot)
                def new_plane(tag):
                    return planes.tile([P, T], dt, name=tag)

                x2 = new_plane("x2")
                y2 = new_plane("y2")
                z2 = new_plane("z2")
                n2a = new_plane("n2a")
                n2 = new_plane("n2")
                xy = new_plane("xy")
                yz = new_plane("yz")
                xz = new_plane("xz")
                x2my2 = new_plane("x2my2")
                t2a = new_plane("t2a")
                a3 = new_plane("a3")
                b3 = new_plane("b3")
                c3 = new_plane("c3")
                d3 = new_plane("d3")
                ya = new_plane("ya")
                xyz = new_plane("xyz")
                yc = new_plane("yc")
                zd = new_plane("zd")
                xc = new_plane("xc")
                zxmy = new_plane("zxmy")
                xb = new_plane("xb")
                sp = planes.tile([P, 4, T], dt, name="sp")

                # squares on ACT (strided reads from xt are fine at 1x)
                nc.scalar.activation(x2, px, F.Square)
                nc.scalar.activation(y2, py, F.Square)
                nc.scalar.activation(z2, pz, F.Square)

                # n2 = x2 + y2 + z2 (DVE)
                nc.vector.tensor_add(n2a, x2, y2)
                nc.vector.tensor_add(n2, n2a, z2)

                # products / derived polys (DVE)
                nc.vector.tensor_mul(xy, px, py)
                nc.vector.tensor_mul(yz, py, pz)
                nc.vector.tensor_mul(xz, px, pz)
                nc.vector.tensor_sub(x2my2, x2, y2)
                # t2a = 3*z2 - n2 = 2z2 - x2 - y2
                nc.vector.scalar_tensor_tensor(
                    t2a, z2, 3.0, n2, op0=ALU.mult, op1=ALU.subtract
                )
                # a3 = 3*x2 - y2
                nc.vector.scalar_tensor_tensor(
                    a3, x2, 3.0, y2, op0=ALU.mult, op1=ALU.subtract
                )
                # b3 = x2 - 3*y2  ==  (-3*y2) + x2
                nc.vector.scalar_tensor_tensor(
                    b3, y2, -3.0, x2, op0=ALU.mult, op1=ALU.add
                )
                # c3 = 5*z2 - n2 = 4z2 - x2 - y2
                nc.vector.scalar_tensor_tensor(
                    c3, z2, 5.0, n2, op0=ALU.mult, op1=ALU.subtract
                )
                # d3 = z2 - 0.6*n2  (so 5*d3 = 2z2 - 3x2 - 3y2; 5 folded into w)
                nc.vector.scalar_tensor_tensor(
                    d3, n2, -0.6, z2, op0=ALU.mult, op1=ALU.add
                )
                nc.vector.tensor_mul(ya, py, a3)
                nc.vector.tensor_mul(xyz, xy, pz)
                nc.vector.tensor_mul(yc, py, c3)
                nc.vector.tensor_mul(zd, pz, d3)
                nc.vector.tensor_mul(xc, px, c3)
                nc.vector.tensor_mul(zxmy, pz, x2my2)
                nc.vector.tensor_mul(xb, px, b3)

                # s'_u = AA1[u] + BB1[u]*n2 (DVE tensor_scalar, 2x mode)
                for u in range(4):
                    nc.vector.tensor_scalar(
                        sp[:, u, :], n2, wap(60 + u), wap(56 + u),
                        op0=ALU.mult, op1=ALU.add,
                    )

                # output tiles: ota = channels 0..31 (m 0..7), otb = 32..63
                ota = opool.tile([P, T * 32], dt, name="ota")
                otb = opool.tile([P, T * 32], dt, name="otb")
                oav = ota.rearrange("p (t g) -> p t g", g=32)
                obv = otb.rearrange("p (t g) -> p t g", g=32)

                # l=0 (m_total 0): out = A0[u] + B0[u]*n2 on ACT
                for u in range(4):
                    nc.scalar.activation(
                        oav[:, :, u], n2, F.Identity,
                        bias=wap(48 + u), scale=wap(52 + u),
                    )

                # l=1 (m_total 1..3): out[m,u] = p_m * s'_u  (order y,z,x)
                ov1 = oav[:, :, 4:12].rearrange("p t (m u) -> p t m u", u=4)
                in0 = xv[:, :, 1:3].unsqueeze(3).broadcast_to([P, T, 2, 4])
                sv = sp.transpose([0, 2, 1])  # [P, T, 4]
                in1 = sv.unsqueeze(2).broadcast_to([P, T, 2, 4])
                nc.vector.tensor_mul(ov1, in0, in1)
                ov1b = oav[:, :, 12:16]
                in0b = px.unsqueeze(2).broadcast_to([P, T, 4])
                nc.vector.tensor_mul(ov1b, in0b, sv)

                # simple groups: out[:, m*4+u] = base * w[m,u]
                group_base = {
                    4: xy, 5: yz, 6: t2a, 7: xz, 8: x2my2,
                    9: ya, 10: xyz, 11: yc, 12: zd, 13: xc, 14: zxmy, 15: xb,
                }
                for m in range(4, 16):
                    base = group_base[m]
                    widx = (m - 4) * 4 if m <= 8 else 20 + (m - 9) * 4
                    if m < 8:
                        ov = oav[:, :, m * 4 : m * 4 + 4]
                    else:
                        ov = obv[:, :, (m - 8) * 4 : (m - 8) * 4 + 4]
                    eng = GROUP_ENGINE[m]
                    if eng == "dve":
                        bvv = base.unsqueeze(2).broadcast_to([P, T, 4])
                        wv = (
                            wc[:, widx : widx + 4]
                            .unsqueeze(1)
                            .broadcast_to([P, T, 4])
                        )
                        nc.vector.tensor_mul(ov, bvv, wv)
                    elif eng == "act":
                        for u in range(4):
                            nc.scalar.activation(
                                ov[:, :, u], base, F.Copy, scale=wap(widx + u)
                            )
                    else:  # gpsimd
                        for u in range(4):
                            nc.gpsimd.tensor_scalar(
                                ov[:, :, u], base, wap(widx + u), None,
                                op0=ALU.mult,
                            )

                nc.sync.dma_start(
                    out=yout[:, ts : ts + T, 0:32],
                    in_=oav,
                )
                nc.sync.dma_start(
                    out=yout[:, ts : ts + T, 32:64],
                    in_=obv,
                )

    nc.compile()
    _cache["nc"] = nc
    return nc


def _host_constants(W0, b0, W1, W2, W3):
    coef2 = np.array([C2_XY, C2_XY, C2_0, C2_XY, C2_2], dtype=np.float64)
    coef3 = np.array(
        [C3_3, C3_2, C3_1, 5.0 * C3_0, C3_1, C3_P2, C3_3], dtype=np.float64
    )
    w2 = coef2[:, None] * W2[0][None, :].astype(np.float64)  # [5, 4]
    w3 = coef3[:, None] * W3[0][None, :].astype(np.float64)  # [7, 4]
    A0 = C0 * W0[0].astype(np.float64) + b0.astype(np.float64)
    B0 = C0 * W0[1].astype(np.float64)
    AA1 = C1 * W1[0].astype(np.float64)
    BB1 = C1 * W1[1].astype(np.float64)
    wconst = np.concatenate(
        [w2.ravel(), w3.ravel(), A0, B0, AA1, BB1]
    ).astype(np.float32)
    assert wconst.shape == (64,)
    return np.ascontiguousarray(np.tile(wconst[None, :], (P, 1)))


def _run(x, W0, b0, W1, W2, W3, trace=False):
    nc = _build_program()
    x = np.ascontiguousarray(np.asarray(x, dtype=np.float32))
    wconst = _host_constants(
        np.asarray(W0, np.float32), np.asarray(b0, np.float32),
        np.asarray(W1, np.float32), np.asarray(W2, np.float32),
        np.asarray(W3, np.float32),
    )
    shards = x.reshape(M_CORES, P, COLS, 3)
    in_maps = [{"xin": shards[c], "wconst": wconst} for c in range(M_CORES)]
    kwargs = {}
    if trace:
        kwargs = dict(trace=True, trace_cores=[0])
    res = run_bass_kernel_spmd(nc, in_maps, list(range(M_CORES)), **kwargs)
    out = np.concatenate(
        [res.results[c]["yout"].reshape(-1, 16, 4) for c in range(M_CORES)],
        axis=0,
    ).reshape(B, N, 16, 4)
    return out, res


def kernel(x, W0, b0, W1, W2, W3):
    out, _ = _run(x, W0, b0, W1, W2, W3)
    return out


def kernel_traced(x, W0, b0, W1, W2, W3):
    """Like kernel(), but also captures an NTFF profile; returns (out, results)."""
    try:
        import axon_profile_shim

        axon_profile_shim.install()
    except Exception:
        import sys
        import types

        if "antenv.axon_hooks" not in sys.modules:
            mod = types.ModuleType("antenv.axon_hooks")
            _h = [None]
            mod.set_axon_ntff_profile_hook = lambda h: _h.__setitem__(0, h)
            mod.get_axon_ntff_profile_hook = lambda: _h[0]
            sys.modules["antenv.axon_hooks"] = mod
            sys.path.insert(0, "/root/.axon_site")
            from trn_agent_boot.trn_boot import _ntff_profile_via_ctypes

            mod.set_axon_ntff_profile_hook(
                _ntff_profile_via_ctypes("/opt/axon/libaxon_pjrt.so")
            )
        import concourse.bass_utils as bu

        bu.upload_artifacts = lambda tmpdir: "local://" + tmpdir
    return _run(x, W0, b0, W1, W2, W3, trace=True)


# revision 4
# speedup vs baseline: 2.2282x; 2.2282x over previous
"""Trainium2 Bass kernel: DGCNN Zernike-monomial interwiner (nn_DGCNN_8839042695322).

Computes, per point p=(x,y,z):
  out[.., 16, 4] = concat_l( einsum(zernike_monoms(p)[l], Wl) ) for l=0..3
Every output channel is a degree<=3 polynomial in (x,y,z); all weights are
folded host-side into per-channel scalar immediates (the compiled program is
cached per weight set). The device computes shared polynomial planes on the
Vector engine and scales them into the interleaved output layout on the
Vector + Scalar engines. Memory-bound: ~33.5 MB HBM traffic per core.

Sharding: pure data parallel over the batch axis across 8 NeuronCores.
"""

import numpy as np

import concourse.bacc as bacc
import concourse.tile as tile
from concourse import mybir
from concourse.bass_utils import run_bass_kernel_spmd

# Problem geometry (hardcoded per spec: x [32, 32768, 3] f32, 8 cores).
B, N, M_CORES = 32, 32768, 8
PTS_PER_CORE = B * N // M_CORES  # 131072
P = 128                          # SBUF partitions
COLS = PTS_PER_CORE // P         # 1024 points per partition
T = 256                          # points per partition per iteration
N_ITERS = COLS // T              # 4

# Real spherical-harmonic constants (match reference).
C0 = 0.28209479177387814
C1 = 0.4886025119029199
C2_XY = 1.0925484305920792
C2_0 = 0.31539156525252005
C2_2 = 0.5462742152960396
C3_3 = 0.5900435899266435
C3_2 = 2.890611442640554
C3_1 = 0.4570457994644658
C3_0 = 0.3731763325901154
C3_P2 = 1.445305721320277

_cache: dict = {}


def _host_constants(W0, b0, W1, W2, W3):
    """Fold interwiner weights into per-channel scalars.

    Returns dict with:
      A0, B0   [4]: l0 channel u = A0[u] + B0[u]*n2
      AA1, BB1 [4]: s'_u = AA1[u] + BB1[u]*n2; l1 channel (m,u) = p_m * s'_u
      w2 [5,4], w3 [7,4]: channel (m,u) = base_m * w[m,u]
    """
    coef2 = np.array([C2_XY, C2_XY, C2_0, C2_XY, C2_2], dtype=np.float64)
    # base for m12 is d3 = z2 - 0.6*n2 = (2z2-3x2-3y2)/5, so fold the 5 in.
    coef3 = np.array(
        [C3_3, C3_2, C3_1, 5.0 * C3_0, C3_1, C3_P2, C3_3], dtype=np.float64
    )
    w2 = (coef2[:, None] * W2[0][None, :].astype(np.float64)).astype(np.float32)
    w3 = (coef3[:, None] * W3[0][None, :].astype(np.float64)).astype(np.float32)
    A0 = (C0 * W0[0].astype(np.float64) + b0.astype(np.float64)).astype(np.float32)
    B0 = (C0 * W0[1].astype(np.float64)).astype(np.float32)
    AA1 = (C1 * W1[0].astype(np.float64)).astype(np.float32)
    BB1 = (C1 * W1[1].astype(np.float64)).astype(np.float32)
    return dict(A0=A0, B0=B0, AA1=AA1, BB1=BB1, w2=w2, w3=w3)


def _build_program(consts):
    dt = mybir.dt.float32
    F = mybir.ActivationFunctionType
    ALU = mybir.AluOpType
    A0, B0 = consts["A0"], consts["B0"]
    AA1, BB1 = consts["AA1"], consts["BB1"]
    w2, w3 = consts["w2"], consts["w3"]

    nc = bacc.Bacc(
        "TRN2", target_bir_lowering=False, debug=False, num_devices=M_CORES
    )
    xin = nc.dram_tensor("xin", [P, COLS, 3], dt, kind="ExternalInput").ap()
    yout = nc.dram_tensor("yout", [P, COLS, 64], dt, kind="ExternalOutput").ap()

    with tile.TileContext(nc) as tc:
        with (
            tc.tile_pool(name="xpool", bufs=2) as xpool,
            tc.tile_pool(name="planes", bufs=2) as planes,
            tc.tile_pool(name="opool", bufs=2) as opool,
        ):
            for it in range(N_ITERS):
                ts = it * T
                xt = xpool.tile([P, T * 3], dt)
                nc.sync.dma_start(
                    out=xt.rearrange("p (t c) -> p t c", c=3),
                    in_=xin[:, ts : ts + T, :],
                )
                xv = xt.rearrange("p (t c) -> p t c", c=3)
                px, py, pz = xv[:, :, 0], xv[:, :, 1], xv[:, :, 2]

                def new_plane(tag):
                    return planes.tile([P, T], dt, name=tag)

                x2 = new_plane("x2")
                y2 = new_plane("y2")
                z2 = new_plane("z2")
                n2a = new_plane("n2a")
                n2 = new_plane("n2")
                xy = new_plane("xy")
                yz = new_plane("yz")
                xz = new_plane("xz")
                x2my2 = new_plane("x2my2")
                t2a = new_plane("t2a")
                a3 = new_plane("a3")
                b3 = new_plane("b3")
                c3 = new_plane("c3")
                d3 = new_plane("d3")
                ya = new_plane("ya")
                xyz = new_plane("xyz")
                yc = new_plane("yc")
                zd = new_plane("zd")
                xc = new_plane("xc")
                zxmy = new_plane("zxmy")
                xb = new_plane("xb")
                sp = planes.tile([P, 4, T], dt, name="sp")

                ot = opool.tile([P, T * 64], dt, name="ot")
                ov = ot.rearrange("p (t g) -> p t g", g=64)

                # squares on ACT (strided reads from xt are fine at 1x)
                nc.scalar.activation(x2, px, F.Square)
                nc.scalar.activation(y2, py, F.Square)
                nc.scalar.activation(z2, pz, F.Square)

                # n2 = x2 + y2 + z2
                nc.vector.tensor_add(n2a, x2, y2)
                nc.vector.tensor_add(n2, n2a, z2)

                # products / derived polys (DVE)
                nc.vector.tensor_mul(xy, px, py)
                nc.vector.tensor_mul(yz, py, pz)
                nc.vector.tensor_mul(xz, px, pz)
                nc.vector.tensor_sub(x2my2, x2, y2)
                # t2a = 3*z2 - n2 = 2z2 - x2 - y2
                nc.vector.scalar_tensor_tensor(
                    t2a, z2, 3.0, n2, op0=ALU.mult, op1=ALU.subtract
                )
                # a3 = 3*x2 - y2
                nc.vector.scalar_tensor_tensor(
                    a3, x2, 3.0, y2, op0=ALU.mult, op1=ALU.subtract
                )
                # b3 = x2 - 3*y2
                nc.vector.scalar_tensor_tensor(
                    b3, y2, -3.0, x2, op0=ALU.mult, op1=ALU.add
                )
                # c3 = 5*z2 - n2 = 4z2 - x2 - y2
                nc.vector.scalar_tensor_tensor(
                    c3, z2, 5.0, n2, op0=ALU.mult, op1=ALU.subtract
                )
                # d3 = z2 - 0.6*n2 (= (2z2-3x2-3y2)/5)
                nc.vector.scalar_tensor_tensor(
                    d3, n2, -0.6, z2, op0=ALU.mult, op1=ALU.add
                )
                nc.vector.tensor_mul(ya, py, a3)
                nc.vector.tensor_mul(xyz, xy, pz)
                nc.vector.tensor_mul(yc, py, c3)
                nc.vector.tensor_mul(zd, pz, d3)
                nc.vector.tensor_mul(xc, px, c3)
                nc.vector.tensor_mul(zxmy, pz, x2my2)
                nc.vector.tensor_mul(xb, px, b3)

                # s'_u = AA1[u] + BB1[u]*n2 (DVE tensor_scalar 2-imm, 2x mode)
                for u in range(4):
                    nc.vector.tensor_scalar(
                        sp[:, u, :], n2, float(BB1[u]), float(AA1[u]),
                        op0=ALU.mult, op1=ALU.add,
                    )

                # l=0 (ch 0..3): out = A0[u] + B0[u]*n2 (DVE TS 2-imm, strided)
                for u in range(4):
                    nc.vector.tensor_scalar(
                        ov[:, :, u], n2, float(B0[u]), float(A0[u]),
                        op0=ALU.mult, op1=ALU.add,
                    )

                # l=1 (ch 4..15): out[m,u] = p_m * s'_u (order y,z,x) on DVE
                for mi, pm in enumerate((py, pz, px)):
                    for u in range(4):
                        nc.vector.tensor_mul(
                            ov[:, :, 4 + mi * 4 + u], pm, sp[:, u, :]
                        )

                # simple groups (ch 16..63): out[m,u] = base_m * w[m,u]
                # m_total -> (base plane, w row, engine)
                groups = [
                    (4, xy, w2[0], "act"),
                    (5, yz, w2[1], "act"),
                    (6, t2a, w2[2], "act"),
                    (7, xz, w2[3], "act"),
                    (8, x2my2, w2[4], "act"),
                    (9, ya, w3[0], "dve"),
                    (10, xyz, w3[1], "act"),
                    (11, yc, w3[2], "dve"),
                    (12, zd, w3[3], "dve"),
                    (13, xc, w3[4], "dve"),
                    (14, zxmy, w3[5], "act"),
                    (15, xb, w3[6], "dve"),
                ]
                for m, base, wrow, eng in groups:
                    for u in range(4):
                        dst = ov[:, :, m * 4 + u]
                        wv = float(wrow[u])
                        if eng == "dve":
                            nc.vector.tensor_scalar(
                                dst, base, wv, None, op0=ALU.mult
                            )
                        else:
                            nc.scalar.activation(dst, base, F.Copy, scale=wv)

                nc.sync.dma_start(out=yout[:, ts : ts + T, :], in_=ov)

    nc.compile()
    return nc


def _get_program(consts):
    key = tuple(
        consts[k].tobytes() for k in ("A0", "B0", "AA1", "BB1", "w2", "w3")
    )
    if _cache.get("key") != key:
        _cache["nc"] = _build_program(consts)
        _cache["key"] = key
    return _cache["nc"]


def _run(x, W0, b0, W1, W2, W3, trace=False):
    consts = _host_constants(
        np.asarray(W0, np.float32), np.asarray(b0, np.float32),
        np.asarray(W1, np.float32), np.asarray(W2, np.float32),
        np.asarray(W3, np.float32),
    )
    nc = _get_program(consts)
    x = np.ascontiguousarray(np.asarray(x, dtype=np.float32))
    shards = x.reshape(M_CORES, P, COLS, 3)
    in_maps = [{"xin": shards[c]} for c in range(M_CORES)]
    kwargs = {}
    if trace:
        kwargs = dict(trace=True, trace_cores=[0])
    res = run_bass_kernel_spmd(nc, in_maps, list(range(M_CORES)), **kwargs)
    out = np.concatenate(
        [res.results[c]["yout"].reshape(-1, 16, 4) for c in range(M_CORES)],
        axis=0,
    ).reshape(B, N, 16, 4)
    return out, res


def kernel(x, W0, b0, W1, W2, W3):
    out, _ = _run(x, W0, b0, W1, W2, W3)
    return out


def kernel_traced(x, W0, b0, W1, W2, W3):
    """Like kernel(), but captures an NTFF profile; returns (out, results)."""
    import sys
    import types

    if "antenv.axon_hooks" not in sys.modules:
        mod = types.ModuleType("antenv.axon_hooks")
        _h = [None]
        mod.set_axon_ntff_profile_hook = lambda h: _h.__setitem__(0, h)
        mod.get_axon_ntff_profile_hook = lambda: _h[0]
        sys.modules["antenv.axon_hooks"] = mod
        if "/root/.axon_site" not in sys.path:
            sys.path.insert(0, "/root/.axon_site")
        from trn_agent_boot.trn_boot import _ntff_profile_via_ctypes

        mod.set_axon_ntff_profile_hook(
            _ntff_profile_via_ctypes("/opt/axon/libaxon_pjrt.so")
        )
    import concourse.bass_utils as bu

    bu.upload_artifacts = lambda tmpdir: "local://" + tmpdir
    return _run(x, W0, b0, W1, W2, W3, trace=True)


# revision 5
# speedup vs baseline: 2.6693x; 1.1979x over previous
"""Trainium2 Bass kernel: DGCNN Zernike-monomial interwiner (nn_DGCNN_8839042695322).

Computes, per point p=(x,y,z):
  out[.., 16, 4] = concat_l( einsum(zernike_monoms(p)[l], Wl) ) for l=0..3
Every output channel is a degree<=3 polynomial in (x,y,z); all weights are
folded host-side into per-channel scalar immediates (the compiled program is
cached per weight set). The device computes shared polynomial planes on the
Vector engine and scales them into the interleaved output layout on the
Vector + Scalar engines. Memory-bound: ~33.5 MB HBM traffic per core.

Sharding: pure data parallel over the batch axis across 8 NeuronCores.
"""

import numpy as np

import concourse.bacc as bacc
import concourse.tile as tile
from concourse import mybir
from concourse.bass_utils import run_bass_kernel_spmd

# Problem geometry (hardcoded per spec: x [32, 32768, 3] f32, 8 cores).
B, N, M_CORES = 32, 32768, 8
PTS_PER_CORE = B * N // M_CORES  # 131072
P = 128                          # SBUF partitions
COLS = PTS_PER_CORE // P         # 1024 points per partition
# iteration lengths: small first (fast first DMA) and last (short tail)
ITER_LENS = [128, 256, 256, 256, 128]
assert sum(ITER_LENS) == COLS

# Real spherical-harmonic constants (match reference).
C0 = 0.28209479177387814
C1 = 0.4886025119029199
C2_XY = 1.0925484305920792
C2_0 = 0.31539156525252005
C2_2 = 0.5462742152960396
C3_3 = 0.5900435899266435
C3_2 = 2.890611442640554
C3_1 = 0.4570457994644658
C3_0 = 0.3731763325901154
C3_P2 = 1.445305721320277

_cache: dict = {}


def _host_constants(W0, b0, W1, W2, W3):
    """Fold interwiner weights into per-channel scalars.

    Returns dict with:
      A0, B0   [4]: l0 channel u = A0[u] + B0[u]*n2
      AA1, BB1 [4]: s'_u = AA1[u] + BB1[u]*n2; l1 channel (m,u) = p_m * s'_u
      w2 [5,4], w3 [7,4]: channel (m,u) = base_m * w[m,u]
    """
    coef2 = np.array([C2_XY, C2_XY, C2_0, C2_XY, C2_2], dtype=np.float64)
    # base for m12 is d3 = z2 - 0.6*n2 = (2z2-3x2-3y2)/5, so fold the 5 in.
    coef3 = np.array(
        [C3_3, C3_2, C3_1, 5.0 * C3_0, C3_1, C3_P2, C3_3], dtype=np.float64
    )
    w2 = (coef2[:, None] * W2[0][None, :].astype(np.float64)).astype(np.float32)
    w3 = (coef3[:, None] * W3[0][None, :].astype(np.float64)).astype(np.float32)
    A0 = (C0 * W0[0].astype(np.float64) + b0.astype(np.float64)).astype(np.float32)
    B0 = (C0 * W0[1].astype(np.float64)).astype(np.float32)
    AA1 = (C1 * W1[0].astype(np.float64)).astype(np.float32)
    BB1 = (C1 * W1[1].astype(np.float64)).astype(np.float32)
    return dict(A0=A0, B0=B0, AA1=AA1, BB1=BB1, w2=w2, w3=w3)


def _build_program(consts):
    dt = mybir.dt.float32
    F = mybir.ActivationFunctionType
    ALU = mybir.AluOpType
    A0, B0 = consts["A0"], consts["B0"]
    AA1, BB1 = consts["AA1"], consts["BB1"]
    w2, w3 = consts["w2"], consts["w3"]

    nc = bacc.Bacc(
        "TRN2", target_bir_lowering=False, debug=False, num_devices=M_CORES
    )
    xin = nc.dram_tensor("xin", [P, COLS, 3], dt, kind="ExternalInput").ap()
    yout = nc.dram_tensor("yout", [P, COLS, 64], dt, kind="ExternalOutput").ap()

    with tile.TileContext(nc) as tc:
        with (
            tc.tile_pool(name="xpool", bufs=1) as xpool,
            tc.tile_pool(name="bases", bufs=2) as bases_pool,
            tc.tile_pool(name="scratch", bufs=1) as scratch,
            tc.tile_pool(name="opool", bufs=2) as opool,
        ):
            # whole input resident: 12KB/partition. Split the load so the
            # first iteration's slice lands fast.
            xall = xpool.tile([P, COLS, 3], dt, name="xall")
            t0 = ITER_LENS[0]
            nc.sync.dma_start(out=xall[:, 0:t0, :], in_=xin[:, 0:t0, :])
            nc.sync.dma_start(out=xall[:, t0:, :], in_=xin[:, t0:, :])

            ts = 0
            for it, T in enumerate(ITER_LENS):
                xv = xall[:, ts : ts + T, :]

                def new_plane(tag, pool=bases_pool):
                    return pool.tile([P, T], dt, name=tag)

                # deinterleave coords (DVE copies; strided read, contig write)
                p3 = scratch.tile([P, 3, T], dt, name="p3")
                nc.vector.tensor_copy(p3.transpose([0, 2, 1]), xv)
                px, py, pz = p3[:, 0, :], p3[:, 1, :], p3[:, 2, :]
                # strided views straight into xall for ACT squares (keeps the
                # Scalar engine independent of the deinterleave)
                sx, sy, sz = xv[:, :, 0], xv[:, :, 1], xv[:, :, 2]

                # ACT-read bases in the double-buffered pool
                x2 = new_plane("x2")
                y2 = new_plane("y2")
                z2 = new_plane("z2")
                xy = new_plane("xy")
                yz = new_plane("yz")
                xz = new_plane("xz")
                x2my2 = new_plane("x2my2")
                t2a = new_plane("t2a")
                xyz = new_plane("xyz")
                zxmy = new_plane("zxmy")
                # DVE-only planes single-buffered
                n2a = new_plane("n2a", scratch)
                n2 = new_plane("n2", scratch)
                a3 = new_plane("a3", scratch)
                b3 = new_plane("b3", scratch)
                c3 = new_plane("c3", scratch)
                d3 = new_plane("d3", scratch)
                ya = new_plane("ya", scratch)
                yc = new_plane("yc", scratch)
                zd = new_plane("zd", scratch)
                xc = new_plane("xc", scratch)
                xb = new_plane("xb", scratch)
                sp = scratch.tile([P, 4, T], dt, name="sp")

                ot = opool.tile([P, T * 64], dt, name="ot")
                ov = ot.rearrange("p (t g) -> p t g", g=64)

                # squares on ACT (strided reads from xall keep ACT decoupled)
                nc.scalar.activation(x2, sx, F.Square)
                nc.scalar.activation(y2, sy, F.Square)
                nc.scalar.activation(z2, sz, F.Square)

                # n2 = x2 + y2 + z2
                nc.vector.tensor_add(n2a, x2, y2)
                nc.vector.tensor_add(n2, n2a, z2)

                # products / derived polys (DVE)
                nc.vector.tensor_mul(xy, px, py)
                nc.vector.tensor_mul(yz, py, pz)
                nc.vector.tensor_mul(xz, px, pz)
                nc.vector.tensor_sub(x2my2, x2, y2)
                # t2a = 3*z2 - n2 = 2z2 - x2 - y2
                nc.vector.scalar_tensor_tensor(
                    t2a, z2, 3.0, n2, op0=ALU.mult, op1=ALU.subtract
                )
                # a3 = 3*x2 - y2
                nc.vector.scalar_tensor_tensor(
                    a3, x2, 3.0, y2, op0=ALU.mult, op1=ALU.subtract
                )
                # b3 = x2 - 3*y2
                nc.vector.scalar_tensor_tensor(
                    b3, y2, -3.0, x2, op0=ALU.mult, op1=ALU.add
                )
                # c3 = 5*z2 - n2 = 4z2 - x2 - y2
                nc.vector.scalar_tensor_tensor(
                    c3, z2, 5.0, n2, op0=ALU.mult, op1=ALU.subtract
                )
                # d3 = z2 - 0.6*n2 (= (2z2-3x2-3y2)/5)
                nc.vector.scalar_tensor_tensor(
                    d3, n2, -0.6, z2, op0=ALU.mult, op1=ALU.add
                )
                nc.vector.tensor_mul(ya, py, a3)
                nc.vector.tensor_mul(xyz, xy, pz)
                nc.vector.tensor_mul(yc, py, c3)
                nc.vector.tensor_mul(zd, pz, d3)
                nc.vector.tensor_mul(xc, px, c3)
                nc.vector.tensor_mul(zxmy, pz, x2my2)
                nc.vector.tensor_mul(xb, px, b3)

                # s'_u = AA1[u] + BB1[u]*n2 (DVE tensor_scalar 2-imm, 2x mode)
                for u in range(4):
                    nc.vector.tensor_scalar(
                        sp[:, u, :], n2, float(BB1[u]), float(AA1[u]),
                        op0=ALU.mult, op1=ALU.add,
                    )

                # l=0 (ch 0..3): out = A0[u] + B0[u]*n2 (DVE TS 2-imm, strided)
                for u in range(4):
                    nc.vector.tensor_scalar(
                        ov[:, :, u], n2, float(B0[u]), float(A0[u]),
                        op0=ALU.mult, op1=ALU.add,
                    )

                # l=1 (ch 4..15): out[m,u] = p_m * s'_u (order y,z,x) on DVE
                for mi, pm in enumerate((py, pz, px)):
                    for u in range(4):
                        nc.vector.tensor_mul(
                            ov[:, :, 4 + mi * 4 + u], pm, sp[:, u, :]
                        )

                # simple groups (ch 16..63): out[m,u] = base_m * w[m,u]
                # m_total -> (base plane, w row, engine)
                groups = [
                    (4, xy, w2[0], "act"),
                    (5, yz, w2[1], "act"),
                    (6, t2a, w2[2], "act"),
                    (7, xz, w2[3], "act"),
                    (8, x2my2, w2[4], "act"),
                    (9, ya, w3[0], "dve"),
                    (10, xyz, w3[1], "act"),
                    (11, yc, w3[2], "dve"),
                    (12, zd, w3[3], "dve"),
                    (13, xc, w3[4], "dve"),
                    (14, zxmy, w3[5], "act"),
                    (15, xb, w3[6], "dve"),
                ]
                for m, base, wrow, eng in groups:
                    for u in range(4):
                        dst = ov[:, :, m * 4 + u]
                        wv = float(wrow[u])
                        if eng == "dve":
                            nc.vector.tensor_scalar(
                                dst, base, wv, None, op0=ALU.mult
                            )
                        else:
                            nc.scalar.activation(dst, base, F.Copy, scale=wv)

                nc.sync.dma_start(out=yout[:, ts : ts + T, :], in_=ov)
                ts += T

    nc.compile()
    return nc


def _get_program(consts):
    key = tuple(
        consts[k].tobytes() for k in ("A0", "B0", "AA1", "BB1", "w2", "w3")
    )
    if _cache.get("key") != key:
        _cache["nc"] = _build_program(consts)
        _cache["key"] = key
    return _cache["nc"]


def _run(x, W0, b0, W1, W2, W3, trace=False):
    consts = _host_constants(
        np.asarray(W0, np.float32), np.asarray(b0, np.float32),
        np.asarray(W1, np.float32), np.asarray(W2, np.float32),
        np.asarray(W3, np.float32),
    )
    nc = _get_program(consts)
    x = np.ascontiguousarray(np.asarray(x, dtype=np.float32))
    shards = x.reshape(M_CORES, P, COLS, 3)
    in_maps = [{"xin": shards[c]} for c in range(M_CORES)]
    kwargs = {}
    if trace:
        kwargs = dict(trace=True, trace_cores=[0])
    res = run_bass_kernel_spmd(nc, in_maps, list(range(M_CORES)), **kwargs)
    out = np.concatenate(
        [res.results[c]["yout"].reshape(-1, 16, 4) for c in range(M_CORES)],
        axis=0,
    ).reshape(B, N, 16, 4)
    return out, res


def kernel(x, W0, b0, W1, W2, W3):
    out, _ = _run(x, W0, b0, W1, W2, W3)
    return out


def kernel_traced(x, W0, b0, W1, W2, W3):
    """Like kernel(), but captures an NTFF profile; returns (out, results)."""
    import sys
    import types

    if "antenv.axon_hooks" not in sys.modules:
        mod = types.ModuleType("antenv.axon_hooks")
        _h = [None]
        mod.set_axon_ntff_profile_hook = lambda h: _h.__setitem__(0, h)
        mod.get_axon_ntff_profile_hook = lambda: _h[0]
        sys.modules["antenv.axon_hooks"] = mod
        if "/root/.axon_site" not in sys.path:
            sys.path.insert(0, "/root/.axon_site")
        from trn_agent_boot.trn_boot import _ntff_profile_via_ctypes

        mod.set_axon_ntff_profile_hook(
            _ntff_profile_via_ctypes("/opt/axon/libaxon_pjrt.so")
        )
    import concourse.bass_utils as bu

    bu.upload_artifacts = lambda tmpdir: "local://" + tmpdir
    return _run(x, W0, b0, W1, W2, W3, trace=True)


# revision 6
# speedup vs baseline: 2.9490x; 1.1048x over previous
"""Trainium2 Bass kernel: DGCNN Zernike-monomial interwiner (nn_DGCNN_8839042695322).

Computes, per point p=(x,y,z):
  out[.., 16, 4] = concat_l( einsum(zernike_monoms(p)[l], Wl) ) for l=0..3
Every output channel is a degree<=3 polynomial in (x,y,z); all weights are
folded host-side into per-channel scalar immediates (the compiled program is
cached per weight set). The device computes shared polynomial planes on the
Vector engine and scales them into the interleaved output layout on the
Vector + Scalar engines. Memory-bound: ~33.5 MB HBM traffic per core.

Sharding: pure data parallel over the batch axis across 8 NeuronCores.
"""

import numpy as np

import concourse.bacc as bacc
import concourse.tile as tile
from concourse import mybir
from concourse.bass_utils import run_bass_kernel_spmd

# Problem geometry (hardcoded per spec: x [32, 32768, 3] f32, 8 cores).
B, N, M_CORES = 32, 32768, 8
PTS_PER_CORE = B * N // M_CORES  # 131072
P = 128                          # SBUF partitions
COLS = PTS_PER_CORE // P         # 1024 points per partition
# iteration lengths: small first (fast first DMA) and last (short tail)
ITER_LENS = [64, 192, 256, 256, 128, 128]
assert sum(ITER_LENS) == COLS

# Real spherical-harmonic constants (match reference).
C0 = 0.28209479177387814
C1 = 0.4886025119029199
C2_XY = 1.0925484305920792
C2_0 = 0.31539156525252005
C2_2 = 0.5462742152960396
C3_3 = 0.5900435899266435
C3_2 = 2.890611442640554
C3_1 = 0.4570457994644658
C3_0 = 0.3731763325901154
C3_P2 = 1.445305721320277

_cache: dict = {}


def _host_constants(W0, b0, W1, W2, W3):
    """Fold interwiner weights into per-channel scalars.

    Returns dict with:
      A0, B0   [4]: l0 channel u = A0[u] + B0[u]*n2
      AA1, BB1 [4]: s'_u = AA1[u] + BB1[u]*n2; l1 channel (m,u) = p_m * s'_u
      w2 [5,4], w3 [7,4]: channel (m,u) = base_m * w[m,u]
    """
    coef2 = np.array([C2_XY, C2_XY, C2_0, C2_XY, C2_2], dtype=np.float64)
    # base for m12 is d3 = z2 - 0.6*n2 = (2z2-3x2-3y2)/5, so fold the 5 in.
    coef3 = np.array(
        [C3_3, C3_2, C3_1, 5.0 * C3_0, C3_1, C3_P2, C3_3], dtype=np.float64
    )
    w2 = (coef2[:, None] * W2[0][None, :].astype(np.float64)).astype(np.float32)
    w3 = (coef3[:, None] * W3[0][None, :].astype(np.float64)).astype(np.float32)
    A0 = (C0 * W0[0].astype(np.float64) + b0.astype(np.float64)).astype(np.float32)
    B0 = (C0 * W0[1].astype(np.float64)).astype(np.float32)
    AA1 = (C1 * W1[0].astype(np.float64)).astype(np.float32)
    BB1 = (C1 * W1[1].astype(np.float64)).astype(np.float32)
    return dict(A0=A0, B0=B0, AA1=AA1, BB1=BB1, w2=w2, w3=w3)


def _build_program(consts):
    dt = mybir.dt.float32
    F = mybir.ActivationFunctionType
    ALU = mybir.AluOpType
    A0, B0 = consts["A0"], consts["B0"]
    AA1, BB1 = consts["AA1"], consts["BB1"]
    w2, w3 = consts["w2"], consts["w3"]

    nc = bacc.Bacc(
        "TRN2", target_bir_lowering=False, debug=False, num_devices=M_CORES
    )
    xin = nc.dram_tensor("xin", [P, COLS, 3], dt, kind="ExternalInput").ap()
    yout = nc.dram_tensor("yout", [P, COLS, 64], dt, kind="ExternalOutput").ap()

    with tile.TileContext(nc) as tc:
        with (
            tc.tile_pool(name="xpool", bufs=1) as xpool,
            tc.tile_pool(name="bases", bufs=2) as bases_pool,
            tc.tile_pool(name="scratch", bufs=1) as scratch,
            tc.tile_pool(name="opool", bufs=2) as opool,
        ):
            # whole input resident: 12KB/partition. Split the load so the
            # first iteration's slice lands fast.
            xall = xpool.tile([P, COLS, 3], dt, name="xall")
            t0 = ITER_LENS[0]
            nc.sync.dma_start(out=xall[:, 0:t0, :], in_=xin[:, 0:t0, :])
            nc.sync.dma_start(out=xall[:, t0:, :], in_=xin[:, t0:, :])

            ts = 0
            for it, T in enumerate(ITER_LENS):
                xv = xall[:, ts : ts + T, :]

                def new_plane(tag, pool=bases_pool):
                    return pool.tile([P, T], dt, name=tag)

                # deinterleave coords (DVE copies; strided read, contig write)
                p3 = scratch.tile([P, 3, T], dt, name="p3")
                nc.vector.tensor_copy(p3.transpose([0, 2, 1]), xv)
                px, py, pz = p3[:, 0, :], p3[:, 1, :], p3[:, 2, :]
                # strided views straight into xall for ACT squares (keeps the
                # Scalar engine independent of the deinterleave)
                sx, sy, sz = xv[:, :, 0], xv[:, :, 1], xv[:, :, 2]

                # ACT-read bases in the double-buffered pool
                x2 = new_plane("x2")
                y2 = new_plane("y2")
                z2 = new_plane("z2")
                xy = new_plane("xy")
                yz = new_plane("yz")
                xz = new_plane("xz")
                x2my2 = new_plane("x2my2")
                t2a = new_plane("t2a")
                xyz = new_plane("xyz")
                zxmy = new_plane("zxmy")
                # DVE-only planes single-buffered
                n2a = new_plane("n2a", scratch)
                n2 = new_plane("n2", scratch)
                a3 = new_plane("a3", scratch)
                b3 = new_plane("b3", scratch)
                c3 = new_plane("c3", scratch)
                d3 = new_plane("d3", scratch)
                ya = new_plane("ya", scratch)
                yc = new_plane("yc", scratch)
                zd = new_plane("zd", scratch)
                xc = new_plane("xc", scratch)
                xb = new_plane("xb", scratch)
                sp = scratch.tile([P, 4, T], dt, name="sp")

                ot = opool.tile([P, T * 64], dt, name="ot")
                ov = ot.rearrange("p (t g) -> p t g", g=64)

                # squares on ACT (strided reads from xall keep ACT decoupled)
                nc.scalar.activation(x2, sx, F.Square)
                nc.scalar.activation(y2, sy, F.Square)
                nc.scalar.activation(z2, sz, F.Square)

                # n2 = x2 + y2 + z2
                nc.vector.tensor_add(n2a, x2, y2)
                nc.vector.tensor_add(n2, n2a, z2)

                # products / derived polys (DVE)
                nc.vector.tensor_mul(xy, px, py)
                nc.vector.tensor_mul(yz, py, pz)
                nc.vector.tensor_mul(xz, px, pz)
                nc.vector.tensor_sub(x2my2, x2, y2)
                # t2a = 3*z2 - n2 = 2z2 - x2 - y2
                nc.vector.scalar_tensor_tensor(
                    t2a, z2, 3.0, n2, op0=ALU.mult, op1=ALU.subtract
                )
                # a3 = 3*x2 - y2
                nc.vector.scalar_tensor_tensor(
                    a3, x2, 3.0, y2, op0=ALU.mult, op1=ALU.subtract
                )
                # b3 = x2 - 3*y2
                nc.vector.scalar_tensor_tensor(
                    b3, y2, -3.0, x2, op0=ALU.mult, op1=ALU.add
                )
                # c3 = 5*z2 - n2 = 4z2 - x2 - y2
                nc.vector.scalar_tensor_tensor(
                    c3, z2, 5.0, n2, op0=ALU.mult, op1=ALU.subtract
                )
                # d3 = z2 - 0.6*n2 (= (2z2-3x2-3y2)/5)
                nc.vector.scalar_tensor_tensor(
                    d3, n2, -0.6, z2, op0=ALU.mult, op1=ALU.add
                )
                nc.vector.tensor_mul(ya, py, a3)
                nc.vector.tensor_mul(xyz, xy, pz)
                nc.vector.tensor_mul(yc, py, c3)
                nc.vector.tensor_mul(zd, pz, d3)
                nc.vector.tensor_mul(xc, px, c3)
                nc.vector.tensor_mul(zxmy, pz, x2my2)
                nc.vector.tensor_mul(xb, px, b3)

                # s'_u = AA1[u] + BB1[u]*n2 (DVE tensor_scalar 2-imm, 2x mode)
                for u in range(4):
                    nc.vector.tensor_scalar(
                        sp[:, u, :], n2, float(BB1[u]), float(AA1[u]),
                        op0=ALU.mult, op1=ALU.add,
                    )

                # l=0 (ch 0..3): out = A0[u] + B0[u]*n2 (DVE TS 2-imm, strided)
                for u in range(4):
                    nc.vector.tensor_scalar(
                        ov[:, :, u], n2, float(B0[u]), float(A0[u]),
                        op0=ALU.mult, op1=ALU.add,
                    )

                # l=1 (ch 4..15): out[m,u] = p_m * s'_u (order y,z,x) on DVE
                for mi, pm in enumerate((py, pz, px)):
                    for u in range(4):
                        nc.vector.tensor_mul(
                            ov[:, :, 4 + mi * 4 + u], pm, sp[:, u, :]
                        )

                # simple groups (ch 16..63): out[m,u] = base_m * w[m,u]
                # m_total -> (base plane, w row, engine)
                groups = [
                    (4, xy, w2[0], "act"),
                    (5, yz, w2[1], "act"),
                    (6, t2a, w2[2], "act"),
                    (7, xz, w2[3], "act"),
                    (8, x2my2, w2[4], "act"),
                    (9, ya, w3[0], "dve"),
                    (10, xyz, w3[1], "act"),
                    (11, yc, w3[2], "dve"),
                    (12, zd, w3[3], "dve"),
                    (13, xc, w3[4], "dve"),
                    (14, zxmy, w3[5], "act"),
                    (15, xb, w3[6], "act"),
                ]
                for m, base, wrow, eng in groups:
                    for u in range(4):
                        dst = ov[:, :, m * 4 + u]
                        wv = float(wrow[u])
                        if eng == "dve":
                            nc.vector.tensor_scalar(
                                dst, base, wv, None, op0=ALU.mult
                            )
                        else:
                            nc.scalar.activation(dst, base, F.Copy, scale=wv)

                nc.sync.dma_start(out=yout[:, ts : ts + T, :], in_=ov)
                ts += T

    nc.compile()
    return nc


def _get_program(consts):
    key = tuple(
        consts[k].tobytes() for k in ("A0", "B0", "AA1", "BB1", "w2", "w3")
    )
    if _cache.get("key") != key:
        _cache["nc"] = _build_program(consts)
        _cache["key"] = key
    return _cache["nc"]


def _run(x, W0, b0, W1, W2, W3, trace=False):
    consts = _host_constants(
        np.asarray(W0, np.float32), np.asarray(b0, np.float32),
        np.asarray(W1, np.float32), np.asarray(W2, np.float32),
        np.asarray(W3, np.float32),
    )
    nc = _get_program(consts)
    x = np.ascontiguousarray(np.asarray(x, dtype=np.float32))
    shards = x.reshape(M_CORES, P, COLS, 3)
    in_maps = [{"xin": shards[c]} for c in range(M_CORES)]
    kwargs = {}
    if trace:
        kwargs = dict(trace=True, trace_cores=[0])
    res = run_bass_kernel_spmd(nc, in_maps, list(range(M_CORES)), **kwargs)
    out = np.concatenate(
        [res.results[c]["yout"].reshape(-1, 16, 4) for c in range(M_CORES)],
        axis=0,
    ).reshape(B, N, 16, 4)
    return out, res


def kernel(x, W0, b0, W1, W2, W3):
    out, _ = _run(x, W0, b0, W1, W2, W3)
    return out


def kernel_traced(x, W0, b0, W1, W2, W3):
    """Like kernel(), but captures an NTFF profile; returns (out, results)."""
    import sys
    import types

    if "antenv.axon_hooks" not in sys.modules:
        mod = types.ModuleType("antenv.axon_hooks")
        _h = [None]
        mod.set_axon_ntff_profile_hook = lambda h: _h.__setitem__(0, h)
        mod.get_axon_ntff_profile_hook = lambda: _h[0]
        sys.modules["antenv.axon_hooks"] = mod
        if "/root/.axon_site" not in sys.path:
            sys.path.insert(0, "/root/.axon_site")
        from trn_agent_boot.trn_boot import _ntff_profile_via_ctypes

        mod.set_axon_ntff_profile_hook(
            _ntff_profile_via_ctypes("/opt/axon/libaxon_pjrt.so")
        )
    import concourse.bass_utils as bu

    bu.upload_artifacts = lambda tmpdir: "local://" + tmpdir
    return _run(x, W0, b0, W1, W2, W3, trace=True)
